# revision 19
# baseline (speedup 1.0000x reference)
"""BERT-base (12-layer, B=8, S=512, D=768, H=12, F=3072) forward pass on 8
Trainium2 NeuronCores.

Strategy: data-parallel over batch (1 sequence per core, no collectives).
Per core, activations are kept feature-major xT[D, S] in SBUF.

v2 over the original baseline:
  - QKV and attention are software-pipelined per head-pair (q/k/v matmuls of
    pair p+1 run on the PE while exp(scores p) runs on ACT) so the tensor
    engine never idles long enough for the HAM clock gate to re-throttle.
  - P@V uses the ones-augmented V column (M=65): softmax denominators fall
    out of the same accumulation, removing 48 M=1 matmuls per layer.
  - LayerNorm gamma/beta are folded into adjacent weights on the host:
    attn_ln (g,b) -> W1', b1'; ffn_ln (g) applied via one fused
    scalar_tensor_tensor pass, (b) absorbed into next-layer QKV biases and
    the attention-residual bias bo'. Normalize is 2 DVE ops per tile.
  - q/k eviction moved from ACT to DVE (ACT is reserved for softmax exp).
"""
import numpy as np

import concourse.bass as bass
import concourse.mybir as mybir
import concourse.tile as tile
from concourse import bass_utils
from concourse.masks import make_identity

AF = mybir.ActivationFunctionType
OP = mybir.AluOpType
F32 = mybir.dt.float32
F32R = mybir.dt.float32r
BF16 = mybir.dt.bfloat16
I32 = mybir.dt.int32

B, S, D, H, F, L, V = 8, 512, 768, 12, 3072, 12, 30522
DK = D // H
SCALE = 1.0 / float(np.sqrt(DK))
NT = D // 128      # 6 feature tiles
NTF = F // 128     # 24 ffn tiles
NST = S // 128     # 4 sequence tiles
NP = H // 2        # 6 head pairs

_NC_CACHE = None


# ---------------------------------------------------------------------------
# wait-slot legalization: walrus codegen allows only ONE sync-wait command on
# TPB instructions; hoist excess waits into standalone EventSemaphores.
def _legalize_waits(nc):
    skip = (mybir.InstEventSemaphore, mybir.InstNoOp)
    n = 0
    for fn in nc.m.functions:
        for blk in fn.blocks:
            out = []
            for inst in blk.instructions:
                si = inst.sync_info
                if si is not None and si.on_wait and not isinstance(inst, skip) \
                        and len(si.on_wait) > 1:
                    waits = list(si.on_wait)
                    for j, w in enumerate(waits[:-1]):
                        ev = mybir.InstEventSemaphore(
                            name=f"{inst.name}-lgw{j}", ins=[], outs=[],
                            sync_info=mybir.SyncInfo(on_wait=[w], on_update=[]),
                        )
                        ev.engine = inst.engine
                        out.append(ev)
                        n += 1
                    inst.sync_info = mybir.SyncInfo(
                        on_wait=[waits[-1]], on_update=list(si.on_update))
                out.append(inst)
            try:
                blk.instructions = out
            except Exception:
                blk.instructions.clear()
                blk.instructions.extend(out)
    return n


def _build_nc():
    nc = bass.Bass("TRN2", target_bir_lowering=False, debug=False,
                   enable_asserts=False, num_devices=8)

    # ---- DRAM I/O ---------------------------------------------------------
    d_ids = nc.dram_tensor("ids", [S, 1], I32, kind="ExternalInput")
    d_tti = nc.dram_tensor("tti", [S, 1], I32, kind="ExternalInput")
    d_mask = nc.dram_tensor("maskadd", [S], F32, kind="ExternalInput")
    d_wemb = nc.dram_tensor("wemb", [V, D], F32, kind="ExternalInput")
    d_pemb = nc.dram_tensor("pemb", [S, D], F32, kind="ExternalInput")
    d_temb = nc.dram_tensor("temb", [2, D], F32, kind="ExternalInput")
    d_eg = nc.dram_tensor("eg", [D], F32, kind="ExternalInput")
    d_eb = nc.dram_tensor("eb", [D], F32, kind="ExternalInput")
    d_wq = nc.dram_tensor("wq", [L, D, D], BF16, kind="ExternalInput")
    d_wk = nc.dram_tensor("wk", [L, D, D], BF16, kind="ExternalInput")
    d_wv = nc.dram_tensor("wv", [L, D, D], BF16, kind="ExternalInput")
    d_wo = nc.dram_tensor("wo", [L, D, D], BF16, kind="ExternalInput")
    d_w1 = nc.dram_tensor("w1", [L, D, F], BF16, kind="ExternalInput")
    d_w2 = nc.dram_tensor("w2", [L, F, D], BF16, kind="ExternalInput")
    d_bq = nc.dram_tensor("bq", [L, D], F32, kind="ExternalInput")
    d_bk = nc.dram_tensor("bk", [L, D], F32, kind="ExternalInput")
    d_bv = nc.dram_tensor("bv", [L, D], F32, kind="ExternalInput")
    d_bo = nc.dram_tensor("bo", [L, D], F32, kind="ExternalInput")
    d_b1 = nc.dram_tensor("b1", [L, F], F32, kind="ExternalInput")
    d_b2 = nc.dram_tensor("b2", [L, D], F32, kind="ExternalInput")
    d_fg = nc.dram_tensor("fg", [L, D], F32, kind="ExternalInput")
    d_fb = nc.dram_tensor("fb", [L, D], F32, kind="ExternalInput")
    d_ones = nc.dram_tensor("ones128", [128], F32, kind="ExternalInput")
    d_onesb = nc.dram_tensor("ones128b", [128], BF16, kind="ExternalInput")
    d_ones512 = nc.dram_tensor("ones512", [1, 512], F32, kind="ExternalInput")
    d_neg1 = nc.dram_tensor("neg1", [1, 128], F32, kind="ExternalInput")
    d_selA = nc.dram_tensor("selA", [1, 128], F32, kind="ExternalInput")
    d_selB = nc.dram_tensor("selB", [1, 128], F32, kind="ExternalInput")
    d_onesgb = nc.dram_tensor("onesgridb", [128, NST * H], BF16, kind="ExternalInput")
    d_out = nc.dram_tensor("out", [S, D], F32, kind="ExternalOutput")

    with tile.TileContext(nc) as tc:
        _emit(nc, tc, locals())
    _legalize_waits(nc)
    return nc


def _emit(nc, tc, d):
    import contextlib
    ctx = contextlib.ExitStack()
    with ctx:
        _emit_body(nc, tc, d, ctx)


def _emit_body(nc, tc, d, ctx):
    pool = ctx.enter_context(tc.tile_pool(name="persist", bufs=1))
    wqkvpool = ctx.enter_context(tc.tile_pool(name="wqkv", bufs=3))
    wpool = ctx.enter_context(tc.tile_pool(name="weights", bufs=3))
    ppool = ctx.enter_context(tc.tile_pool(name="params", bufs=2))
    epool = ctx.enter_context(tc.tile_pool(name="epool", bufs=10))
    hpool = ctx.enter_context(tc.tile_pool(name="hpool", bufs=4))
    spool = ctx.enter_context(tc.tile_pool(name="smalls", bufs=1))

    # ---- persistent constants --------------------------------------------
    ones_col = pool.tile([128, 1], F32R, name="ones_col")
    nc.sync.dma_start(ones_col[:], d["d_ones"].ap().rearrange("(p o) -> p o", o=1).bitcast(F32R))
    ones_colb = pool.tile([128, 1], BF16, name="ones_colb")
    nc.sync.dma_start(ones_colb[:], d["d_onesb"].ap().rearrange("(p o) -> p o", o=1))
    one_row = pool.tile([1, 128], F32R, name="one_row")
    nc.sync.dma_start(one_row[:], d["d_ones"].ap().rearrange("(o p) -> o p", o=1).bitcast(F32R))
    ones_s = pool.tile([1, 512], F32R, name="ones_s")
    nc.sync.dma_start(ones_s[:], d["d_ones512"].ap()[:, :].bitcast(F32R))
    neg_row = pool.tile([1, 128], F32R, name="neg_row")
    nc.sync.dma_start(neg_row[:], d["d_neg1"].ap()[:, :].bitcast(F32R))
    selA = pool.tile([1, 128], F32R, name="selA")
    nc.sync.dma_start(selA[:], d["d_selA"].ap()[:, :].bitcast(F32R))
    selB = pool.tile([1, 128], F32R, name="selB")
    nc.sync.dma_start(selB[:], d["d_selB"].ap()[:, :].bitcast(F32R))
    ident = pool.tile([128, 128], F32, name="ident")
    make_identity(nc, ident[:])
    ident16 = pool.tile([128, 128], BF16, name="ident16")
    make_identity(nc, ident16[:])
    eps5 = pool.tile([1, 1], F32, name="eps5")
    nc.vector.memset(eps5[:], 1e-5)
    eps12 = pool.tile([1, 1], F32, name="eps12")
    nc.vector.memset(eps12[:], 1e-12)
    maskc = pool.tile([128, NST], F32, name="maskc")
    nc.sync.dma_start(maskc[:], d["d_mask"].ap().rearrange("(n p) -> p n", p=128))

    # ---- persistent activations ------------------------------------------
    xT = pool.tile([128, NT, S], BF16, name="xT")       # layer input, feature-major
    aT = pool.tile([128, NT, S], BF16, name="aT")       # post-attn LN out
    qkT = pool.tile([128, NT, 2, S], BF16, name="qkT")  # q at [...,0,:], k at [...,1,:]
    cT = pool.tile([128, NT, S], BF16, name="cT")       # ctx, feature-major
    ybuf = pool.tile([128, NT, S], F32R, name="ybuf")   # pre-LN staging
    vaug = pool.tile([128, NST, H, DK + 1], BF16, name="vaug")
    # ones column of vaug (written once)
    nc.sync.dma_start(
        vaug[:, :, :, DK:DK + 1],
        d["d_onesgb"].ap().rearrange("p (a b) -> p a b", a=NST)[:, :, :],
    )

    # =======================================================================
    # layernorm along the feature (partition-spread) dim, feature-major.
    # y: [128, nt, S] F32R tile. Normalize:
    #   out = (y - mu) * rstd            (g_col is None)
    #   out = ((y - mu) * g) * rstd      (g_col given, fused via stt)
    # optional extra per-partition bias b_col added afterwards (final layer /
    # embedding). Writes out[:, dt, :].
    def layernorm(y, nt, out, psum_pool, eps_col, dim, g_col=None, b_col=None):
        # squares (for E[y^2]) start immediately on ACT; mean path on PE/DVE
        sq_t = []
        for dt in range(nt):
            sqt = spool.tile([128, S], BF16, name=f"sq{dt}", tag=f"sq{dt}")
            nc.scalar.activation(sqt[:], y[:, dt, :].bitcast(F32), AF.Square)
            sq_t.append(sqt)
        s0 = psum_pool.tile([1, S], F32, name="s0", tag="st0")
        s1 = psum_pool.tile([1, S], F32, name="s1", tag="st1")
        for dt in range(nt):
            nc.tensor.matmul(s0[:], ones_col[:], y[:, dt, :],
                             start=(dt == 0), stop=(dt == nt - 1))
        # stats chain on ACT/GPSIMD: the DVE queue is backlogged with the
        # previous phase's evictions at LayerNorm entry, ACT/GPSIMD are idle
        mu = spool.tile([1, S], F32R, name="mu", tag="ln_mu")
        nc.scalar.activation(mu[:], s0[:], AF.Identity, scale=1.0 / dim)
        for dt in range(nt):
            nc.tensor.matmul(s1[:], ones_colb[:], sq_t[dt][:],
                             start=(dt == 0), stop=(dt == nt - 1))
        msq = spool.tile([1, S], F32, name="msq", tag="ln_msq")
        nc.scalar.activation(msq[:], s1[:], AF.Identity, scale=1.0 / dim)
        musq = spool.tile([1, S], F32, name="musq", tag="ln_musq")
        nc.scalar.activation(musq[:], mu[:].bitcast(F32), AF.Square)
        var = spool.tile([1, S], F32R, name="var", tag="ln_var")
        nc.gpsimd.tensor_tensor(var[:], msq[:], musq[:], op=OP.subtract)
        negmu_ps = psum_pool.tile([128, S], F32, name="negmu_ps", tag="bc0")
        nc.tensor.matmul(negmu_ps[:], neg_row[:], mu[:], start=True, stop=True)
        # warm-keepers: tiny dependent matmuls so the PE array stays active
        # through the var -> rstd chain (keeps the HAM clock gate at 8/8)
        warm_ps = psum_pool.tile([128, S], F32, name="warm_ps", tag="warm")
        nc.tensor.matmul(warm_ps[:], one_row[:], var[:], start=True, stop=True)
        lnv = spool.tile([1, S], F32, name="lnv", tag="ln_lnv")
        nc.scalar.activation(lnv[:], var[:].bitcast(F32), AF.Ln, bias=eps_col[:, 0:1])
        rstd = spool.tile([1, S], F32R, name="rstd", tag="ln_rstd")
        nc.scalar.activation(rstd[:], lnv[:], AF.Exp, scale=-0.5)
        # pass 1 (in-place, overlaps the rstd chain): y -= mu
        nc.vector.tensor_tensor(y[:, 0, :], y[:, 0, :].bitcast(F32),
                                negmu_ps[:], op=OP.add)
        nc.tensor.matmul(warm_ps[:], one_row[:], rstd[:], start=True, stop=True)
        rstd_ps = psum_pool.tile([128, S], F32, name="rstd_ps", tag="bc1")
        nc.tensor.matmul(rstd_ps[:], one_row[:], rstd[:], start=True, stop=True)
        for dt in range(nt):
            if g_col is not None:
                nc.vector.scalar_tensor_tensor(
                    out[:, dt, :], y[:, dt, :].bitcast(F32),
                    g_col[:, dt:dt + 1], rstd_ps[:], op0=OP.mult, op1=OP.mult)
            else:
                nc.vector.tensor_tensor(out[:, dt, :], y[:, dt, :].bitcast(F32),
                                        rstd_ps[:], op=OP.mult)
            if b_col is not None:
                nc.vector.tensor_scalar(out[:, dt, :], out[:, dt, :],
                                        b_col[:, dt:dt + 1], None, OP.add)
            # warm-keeper during the normalize tail
            nc.tensor.matmul(warm_ps[0:1, :], ones_colb[:], out[:, dt, :],
                             start=True, stop=True)
            if dt + 1 < nt:
                nc.vector.tensor_tensor(y[:, dt + 1, :], y[:, dt + 1, :].bitcast(F32),
                                        negmu_ps[:], op=OP.add)

    # =======================================================================
    # embedding: gather + add + transpose to feature-major + LN -> xT
    with (
        tc.tile_pool(name="emb_sb", bufs=2) as embp,
        tc.tile_pool(name="emb_ps", bufs=3, space="PSUM") as embps,
    ):
        egc = ppool.tile([128, NT], F32, name="egc")
        nc.sync.dma_start(egc[:], d["d_eg"].ap().rearrange("(n p) -> p n", p=128))
        ebc = ppool.tile([128, NT], F32, name="ebc")
        nc.sync.dma_start(ebc[:], d["d_eb"].ap().rearrange("(n p) -> p n", p=128))
        for st in range(NST):
            idst = embp.tile([128, 1], I32, name="idst", tag="idst")
            nc.sync.dma_start(idst[:], d["d_ids"].ap()[st * 128:(st + 1) * 128, :])
            ttst = embp.tile([128, 1], I32, name="ttst", tag="ttst")
            nc.sync.dma_start(ttst[:], d["d_tti"].ap()[st * 128:(st + 1) * 128, :])
            x0 = embp.tile([128, D], F32, name="x0", tag="x0")
            nc.gpsimd.indirect_dma_start(
                out=x0[:], out_offset=None, in_=d["d_wemb"].ap(),
                in_offset=bass.IndirectOffsetOnAxis(ap=idst[:, :1], axis=0))
            tg = embp.tile([128, D], F32, name="tg", tag="tg")
            nc.gpsimd.indirect_dma_start(
                out=tg[:], out_offset=None, in_=d["d_temb"].ap(),
                in_offset=bass.IndirectOffsetOnAxis(ap=ttst[:, :1], axis=0))
            pg = embp.tile([128, D], F32, name="pg", tag="pg")
            nc.sync.dma_start(pg[:], d["d_pemb"].ap()[st * 128:(st + 1) * 128, :])
            nc.vector.tensor_tensor(x0[:], x0[:], tg[:], op=OP.add)
            nc.vector.tensor_tensor(x0[:], x0[:], pg[:], op=OP.add)
            for dt in range(NT):
                trp = embps.tile([128, 128], F32, name="trp", tag="trp")
                nc.tensor.transpose(trp[:], x0[:, dt * 128:(dt + 1) * 128], ident[:])
                nc.vector.tensor_copy(ybuf[:, dt, st * 128:(st + 1) * 128], trp[:])
        with tc.tile_pool(name="eln_ps", bufs=1, space="PSUM") as elnps:
            layernorm(ybuf, NT, xT, elnps, eps12, D, g_col=egc[:], b_col=ebc[:])

    # =======================================================================
    # transformer layers
    for l in range(L):
        # ---- per-layer params -------------------------------------------
        bqr = ppool.tile([1, D], F32R, name="bqr", tag="bqr")
        nc.sync.dma_start(bqr[:], d["d_bq"].ap()[l].rearrange("(o e) -> o e", o=1).bitcast(F32R))
        bkr = ppool.tile([1, D], F32R, name="bkr", tag="bkr")
        nc.sync.dma_start(bkr[:], d["d_bk"].ap()[l].rearrange("(o e) -> o e", o=1).bitcast(F32R))
        bvc = ppool.tile([128, NT], F32, name="bvc", tag="bvc")
        nc.sync.dma_start(bvc[:], d["d_bv"].ap()[l].rearrange("(n p) -> p n", p=128))
        bor = ppool.tile([1, D], F32R, name="bor", tag="bor")
        nc.sync.dma_start(bor[:], d["d_bo"].ap()[l].rearrange("(o e) -> o e", o=1).bitcast(F32R))
        b1c = ppool.tile([128, NTF], F32, name="b1c", tag="b1c")
        nc.sync.dma_start(b1c[:], d["d_b1"].ap()[l].rearrange("(n p) -> p n", p=128))
        b2c = ppool.tile([128, NT], F32, name="b2c", tag="b2c")
        nc.sync.dma_start(b2c[:], d["d_b2"].ap()[l].rearrange("(n p) -> p n", p=128))
        fgc = ppool.tile([128, NT], F32, name="fgc", tag="fgc")
        nc.sync.dma_start(fgc[:], d["d_fg"].ap()[l].rearrange("(n p) -> p n", p=128))
        fbc = ppool.tile([128, NT], F32, name="fbc", tag="fbc")
        nc.sync.dma_start(fbc[:], d["d_fb"].ap()[l].rearrange("(n p) -> p n", p=128))

        # ---- phase 1+2: QKV / attention pair-pipeline --------------------
        # stage(p):
        #   1. PV(p-1) dense at stage start (e tiles + vaug are a full stage
        #      old) -> softmax denominators early; recip chain (merged both
        #      heads) overlaps the rest of the stage.
        #   2. scores(p) (+ exp on ACT) interleaved with q/k/v(p+1) matmuls.
        #   3. tail: rps/rsb/cT for p-1 mid-stage, q/k/v eviction+transpose.
        # This keeps the in-order PE queue dense (no ACT round-trips on the
        # critical path) so the HAM clock gate stays at 8/8.
        with (
            tc.tile_pool(name="a_ps", bufs=1, space="PSUM") as aps,
            tc.tile_pool(name="sc_ps", bufs=3, space="PSUM") as scps,
            tc.tile_pool(name="ctx_ps", bufs=1, space="PSUM") as ctxps,
        ):
            e_tiles = [None] * NP

            def emit_qkv(p):
                wqt = wqkvpool.tile([128, NT, 128], BF16, name=f"wq{p}", tag="wq")
                wkt = wqkvpool.tile([128, NT, 128], BF16, name=f"wk{p}", tag="wk")
                wvt = wqkvpool.tile([128, NT, 128], BF16, name=f"wv{p}", tag="wv")
                for (wt, wd) in ((wqt, d["d_wq"]), (wkt, d["d_wk"]), (wvt, d["d_wv"])):
                    for g3 in range(3):
                        nc.sync.dma_start(
                            wt[:, 2 * g3:2 * g3 + 2, :],
                            wd.ap()[l, g3 * 256:(g3 + 1) * 256, p * 128:(p + 1) * 128]
                            .rearrange("(n p) e -> p n e", p=128))
                qkacc = aps.tile([128, 2, S], F32, name=f"qka{p}", tag="qkacc")
                vacc = aps.tile([128, S], F32, name=f"va{p}", tag="vacc")

                def chunk(dt):
                    nc.tensor.matmul(qkacc[:, 0, :], wqt[:, dt, :], xT[:, dt, :],
                                     start=(dt == 0), stop=False)
                    nc.tensor.matmul(qkacc[:, 1, :], wkt[:, dt, :], xT[:, dt, :],
                                     start=(dt == 0), stop=False)
                    nc.tensor.matmul(vacc[:], wvt[:, dt, :], xT[:, dt, :],
                                     start=(dt == 0), stop=(dt == NT - 1))

                def tail():
                    # biases as rank-1 matmuls, then one fused q+k eviction
                    nc.tensor.matmul(qkacc[:, 0, :], bqr[:, p * 128:(p + 1) * 128],
                                     ones_s[:], start=False, stop=True,
                                     skip_group_check=True)
                    nc.tensor.matmul(qkacc[:, 1, :], bkr[:, p * 128:(p + 1) * 128],
                                     ones_s[:], start=False, stop=True,
                                     skip_group_check=True)
                    nc.vector.tensor_copy(qkT[:, p, :, :], qkacc[:, :, :])
                    # v feature-major -> SBUF (+bias per-feature) -> transpose
                    vfm = hpool.tile([128, S], BF16, name=f"vfm{p}", tag="vfm",
                                     bufs=2)
                    nc.vector.tensor_scalar(vfm[:], vacc[:], bvc[:, p:p + 1],
                                            None, OP.add)
                    vt = scps.tile([128, NST, 128], BF16, name=f"vt{p}", tag="sc")
                    for st in range(NST):
                        nc.tensor.transpose(vt[:, st, :],
                                            vfm[:, st * 128:(st + 1) * 128],
                                            ident16[:])
                    nc.vector.tensor_copy(
                        vaug[:, :, 2 * p:2 * p + 2, 0:DK],
                        vt[:, :, :].rearrange("p (a b) e -> p a b e", b=2)[:, :, :, :])

                return chunk, tail

            def stage(p, qkv_next):
                # --- 1. PV block for pair p-1 (dense) ---
                cps = None
                if p >= 1:
                    prev_ets = e_tiles[p - 1]
                    cps = ctxps.tile([DK + 1, 2, S], F32, name=f"cps{p-1}",
                                     tag="ctx")
                    for kt in range(NST):
                        for hh in range(2):
                            nc.tensor.matmul(
                                cps[:, hh, :],
                                vaug[:, kt, 2 * (p - 1) + hh, 0:DK + 1],
                                prev_ets[kt * 2 + hh][:],
                                start=(kt == 0), stop=(kt == NST - 1))
                    # merged denominator -> reciprocal for both heads
                    nlden = spool.tile([1, 2, S], F32, name="nld", tag="nlden")
                    nc.scalar.activation(nlden[:], cps[DK:DK + 1, :, :], AF.Ln)
                    recip = spool.tile([1, 2, S], F32R, name="rcp", tag="recip")
                    nc.scalar.activation(recip[:], nlden[:], AF.Exp, scale=-1.0)
                # --- 2. scores(p) + qkv(p+1), interleaved ---
                ets = [] if p < NP else None
                for j in range(8):
                    kt, hh = divmod(j, 2)
                    if p < NP:
                        lo = hh * 64
                        sc = scps.tile([128, S], F32, name=f"sc{kt}{hh}", tag="sc")
                        nc.tensor.matmul(
                            sc[:], qkT[lo:lo + 64, p, 1, kt * 128:(kt + 1) * 128],
                            qkT[lo:lo + 64, p, 0, :], start=True, stop=True)
                        et = epool.tile([128, S], BF16, name=f"e{kt}{hh}", tag="e")
                        nc.scalar.activation(et[:], sc[:], AF.Exp,
                                             bias=maskc[:, kt:kt + 1])
                        ets.append(et)
                    if qkv_next is not None and j < NT:
                        qkv_next[0](j)
                    if p >= 1 and j == 4:
                        # p-1 epilogue mid-stage: recip is ready by now
                        rps = scps.tile([128, S], F32, name="rps", tag="sc")
                        nc.tensor.matmul(rps[:], selA[:], recip[:, 0, :],
                                         start=True, stop=False)
                        nc.tensor.matmul(rps[:], selB[:], recip[:, 1, :],
                                         start=False, stop=True,
                                         skip_group_check=True)
                        rsb = spool.tile([128, S], F32, name="rsb", tag="rsb")
                        nc.vector.tensor_copy(rsb[:], rps[:])
                        for hh2 in range(2):
                            lo2 = hh2 * 64
                            nc.vector.tensor_tensor(cT[lo2:lo2 + DK, p - 1, :],
                                                    cps[0:DK, hh2, :],
                                                    rsb[lo2:lo2 + DK, :],
                                                    op=OP.mult)
                if p < NP:
                    e_tiles[p] = ets
                if qkv_next is not None:
                    qkv_next[1]()

            # prologue: q/k/v of pair 0 (dt-chunked, consumes xT in arrival
            # order off the preceding LayerNorm)
            qkv0 = emit_qkv(0)
            for dt in range(NT):
                qkv0[0](dt)
            qkv0[1]()
            for p in range(NP + 1):
                stage(p, emit_qkv(p + 1) if p + 1 < NP else None)

        # ---- phase 3: Wo + residual -> ybuf ------------------------------
        with tc.tile_pool(name="wo_ps", bufs=3, space="PSUM") as wops:
            for et in range(NT):
                wt = wpool.tile([128, NT, 128], BF16, name=f"wo{et}", tag="wo")
                for g3 in range(3):
                    nc.sync.dma_start(
                        wt[:, 2 * g3:2 * g3 + 2, :],
                        d["d_wo"].ap()[l, g3 * 256:(g3 + 1) * 256, et * 128:(et + 1) * 128]
                        .rearrange("(n p) e -> p n e", p=128))
                acc = wops.tile([128, S], F32, name=f"o{et}", tag="acc")
                for dt in range(NT):
                    nc.tensor.matmul(acc[:], wt[:, dt, :], cT[:, dt, :],
                                     start=(dt == 0), stop=False)
                nc.tensor.matmul(acc[:], bor[:, et * 128:(et + 1) * 128],
                                 ones_s[:], start=False, stop=True,
                                 skip_group_check=True)
                nc.vector.tensor_tensor(ybuf[:, et, :], acc[:],
                                        xT[:, et, :], op=OP.add)

        # ---- LN1 -> aT (gamma/beta folded into W1'/b1') ------------------
        with tc.tile_pool(name="ln1_ps", bufs=1, space="PSUM") as lnps:
            layernorm(ybuf, NT, aT, lnps, eps5, D)

        # ---- phase 4: FFN (W1 -> h, W2 accumulate into 6 yT banks) -------
        with (
            tc.tile_pool(name="y_ps", bufs=1, space="PSUM") as yps,
            tc.tile_pool(name="h_ps", bufs=2, space="PSUM") as hps,
        ):
            ytiles = [yps.tile([128, S], F32, name=f"yt{et}", tag=f"y{et}")
                      for et in range(NT)]
            h_sb = [None] * NTF

            def load_w1w2(c):
                w1t = wpool.tile([128, NT, 256], BF16, name=f"w1_{c}", tag="w1")
                for g3 in range(3):
                    nc.sync.dma_start(
                        w1t[:, 2 * g3:2 * g3 + 2, :],
                        d["d_w1"].ap()[l, g3 * 256:(g3 + 1) * 256, c * 256:(c + 1) * 256]
                        .rearrange("(n p) e -> p n e", p=128))
                w2t = wpool.tile([128, 2, D], BF16, name=f"w2_{c}", tag="w2")
                for g2 in range(2):
                    nc.sync.dma_start(
                        w2t[:, g2:g2 + 1, :],
                        d["d_w2"].ap()[l, c * 256 + g2 * 128:c * 256 + (g2 + 1) * 128, :]
                        .rearrange("(n p) e -> p n e", p=128))
                return w1t, w2t

            def emit_h(f):
                c, fj = divmod(f, 2)
                if fj == 0:
                    emit_h.w1t, w2t = load_w1w2(c)
                    emit_h.w2t = w2t
                hacc = hps.tile([128, S], F32, name=f"h{f}", tag="hacc")
                for dt in range(NT):
                    nc.tensor.matmul(hacc[:], emit_h.w1t[:, dt, fj * 128:(fj + 1) * 128],
                                     aT[:, dt, :], start=(dt == 0), stop=(dt == NT - 1))
                hs = hpool.tile([128, S], BF16, name=f"hs{f}", tag="hs")
                nc.scalar.activation(hs[:], hacc[:], AF.Identity, bias=b1c[:, f:f + 1])
                h_sb[f] = hs
                emit_h.w2ts[f] = emit_h.w2t

            def emit_y(f):
                fj = f % 2
                w2t = emit_h.w2ts[f]
                for et in range(NT):
                    nc.tensor.matmul(ytiles[et][:],
                                     w2t[:, fj, et * 128:(et + 1) * 128],
                                     h_sb[f][:], start=(f == 0), stop=(f == NTF - 1))
                h_sb[f] = None

            emit_h.w2ts = [None] * NTF
            emit_h(0)
            for f in range(1, NTF):
                emit_h(f)
                emit_y(f - 1)
            emit_y(NTF - 1)

            # epilogue: +b2 (per-partition) -> ybuf
            for et in range(NT):
                nc.vector.tensor_scalar(ybuf[:, et, :], ytiles[et][:],
                                        b2c[:, et:et + 1], None, OP.add)

        # ---- LN2 -> xT (next layer input) --------------------------------
        # fg applied via fused stt; fb folded into next layer's biases,
        # except for the final layer where it is applied explicitly.
        with tc.tile_pool(name="ln2_ps", bufs=1, space="PSUM") as lnps:
            layernorm(ybuf, NT, xT, lnps, eps5, D, g_col=fgc[:],
                      b_col=(fbc[:] if l == L - 1 else None))

    # =======================================================================
    # output: transpose xT -> [S, D] and DMA out
    with (
        tc.tile_pool(name="out_sb", bufs=2) as outp,
        tc.tile_pool(name="out_ps", bufs=2, space="PSUM") as outps,
    ):
        for st in range(NST):
            ops_t = outps.tile([128, D], BF16, name="ops", tag="ops")
            for dt in range(NT):
                nc.tensor.transpose(ops_t[:, dt * 128:(dt + 1) * 128],
                                    xT[:, dt, st * 128:(st + 1) * 128],
                                    ident16[:])
            osb = outp.tile([128, D], F32, name="osb", tag="osb")
            nc.vector.tensor_copy(osb[:], ops_t[:])
            nc.sync.dma_start(d["d_out"].ap()[st * 128:(st + 1) * 128, :], osb[:])


# ---------------------------------------------------------------------------
def kernel(**inputs):
    global _NC_CACHE
    if _NC_CACHE is None:
        _NC_CACHE = _build_nc()
    nc = _NC_CACHE

    import ml_dtypes
    f32 = lambda a: np.ascontiguousarray(np.asarray(a), dtype=np.float32)
    bf = lambda a: np.ascontiguousarray(a.astype(ml_dtypes.bfloat16))

    Wq = f32(inputs["Wq"])
    Wk = f32(inputs["Wk"])
    Wv = f32(inputs["Wv"])
    W1 = f32(inputs["W1"])
    ag = f32(inputs["attn_ln_g"])
    ab = f32(inputs["attn_ln_b"])
    fg = f32(inputs["ffn_ln_g"])
    fb = f32(inputs["ffn_ln_b"])

    # LN folding (host):
    #  - attn_ln:  a = ag*ahat + ab feeds only W1:
    #      W1' = W1 * ag[None,:],  b1' = b1 + W1 @ ab
    #  - ffn_ln of layer l-1: x = fg*xhat + fb; kernel materializes fg*xhat:
    #      bq'[l] = bq[l] + Wq[l] @ fb[l-1]   (same for k, v)
    #      bo'[l] = bo[l] + fb[l-1]
    fb_prev = np.concatenate([np.zeros((1, D), np.float32), fb[:-1]], axis=0)
    W1p = W1 * ag[:, None, :]
    b1p = f32(inputs["b1"]) + np.einsum("lfd,ld->lf", W1, ab)
    bqp = (f32(inputs["bq"]) + np.einsum("led,ld->le", Wq, fb_prev)) * SCALE
    bkp = f32(inputs["bk"]) + np.einsum("led,ld->le", Wk, fb_prev)
    bvp = f32(inputs["bv"]) + np.einsum("led,ld->le", Wv, fb_prev)
    bop = f32(inputs["bo"]) + fb_prev

    shared = {
        "wemb": f32(inputs["word_emb"]),
        "pemb": f32(inputs["pos_emb"])[:S],
        "temb": f32(inputs["type_emb"]),
        "eg": f32(inputs["emb_ln_g"]), "eb": f32(inputs["emb_ln_b"]),
        "wq": bf((Wq * SCALE).transpose(0, 2, 1)),
        "wk": bf(Wk.transpose(0, 2, 1)),
        "wv": bf(Wv.transpose(0, 2, 1)),
        "wo": bf(f32(inputs["Wo"]).transpose(0, 2, 1)),
        "w1": bf(W1p.transpose(0, 2, 1)),
        "w2": bf(f32(inputs["W2"]).transpose(0, 2, 1)),
        "bq": bqp, "bk": bkp, "bv": bvp,
        "bo": bop, "b1": b1p, "b2": f32(inputs["b2"]),
        "fg": fg, "fb": fb,
        "ones128": np.ones(128, np.float32),
        "ones128b": np.ones(128, ml_dtypes.bfloat16),
        "ones512": np.ones((1, 512), np.float32),
        "neg1": np.full((1, 128), -1.0, np.float32),
        "selA": np.concatenate([np.ones((1, 64)), np.zeros((1, 64))], 1).astype(np.float32),
        "selB": np.concatenate([np.zeros((1, 64)), np.ones((1, 64))], 1).astype(np.float32),
        "onesgridb": np.ones((128, NST * H), ml_dtypes.bfloat16),
    }
    ids = np.asarray(inputs["input_ids"]).astype(np.int32)
    tti = np.asarray(inputs["token_type_ids"]).astype(np.int32)
    am = np.asarray(inputs["attention_mask"]).astype(np.float32)
    in_maps = []
    for c in range(B):
        in_maps.append({
            **shared,
            "ids": ids[c].reshape(S, 1),
            "tti": tti[c].reshape(S, 1),
            "maskadd": np.where(am[c] == 0, -1e9, 0.0).astype(np.float32),
        })
    res = bass_utils.run_bass_kernel_spmd(
        nc, in_maps, core_ids=list(range(B)), trace=False)
    out = np.stack([res.results[c]["out"] for c in range(B)], axis=0)
    return out.astype(np.float32)
